# revision 34
# baseline (speedup 1.0000x reference)
"""NonLocalAttention Trainium2 kernel (v2 — row-tiled QK, pipelined softmax).

Reference computation (N=2, C=64, CR=32, H=W=96, HW=9216):
    e1  = PReLU(w1 @ inputa + b1)   # [N,32,HW]   (queries)
    e2  = PReLU(w2 @ inputb + b2)   # [N,32,HW]   (keys)
    asm = PReLU(wa @ inputa + ba)   # [N,64,HW]   (values)
    out = softmax(e1^T e2, axis=keys) @ asm^T + inputa

Sharding: 8 cores = 2 batches x 4 query-chunks of 2304 rows. Softmax is
key-order invariant, so the host ROTATES the key/value columns per core so
that each core's query chunk is always columns 0:2304 — one SPMD program,
no per-core offsets, and no separate xq input. No collectives.

Per-core kernel (flash-style, never materializes [HW,HW]):
  - QK is ROW-TILED: the contraction is only CR=32, so three K=32 matmuls
    run CONCURRENTLY in PE row-groups 0..2 (tile_position via base_partition),
    each producing S^T for one 128-key tile. ~2.4x faster than one K=128
    matmul and needs no zero-padding of e1/e2.
  - e1 is computed with a column-replicated stationary w1r [128,128] (4
    copies of w1^T+bias) so the PReLU output lands replicated in all four
    32-partition groups, ready to be the row-tiled QK moving operand.
  - e2 is computed with COL-TILED matmuls (stationary w2 at col-group j)
    so key tile 3g+j lands directly at partitions 32j, col block g.
  - conv biases fold into the matmuls via an augmented ones-row (row 64);
    moving operands are [65,HW] from the host plus one DVE memset of rows
    65:128 (keeps K=128 full-rate contraction for the convs).
  - attention loop is software-pipelined: QK(g+1) is emitted BEFORE PV(g)
    so the tensor engine never waits on exp(g); the PV accumulator po is
    double-buffered so the softmax-divide epilogue of block b overlaps
    block b+1 (the baseline stalled 6-9us per block here, which also
    re-throttled the PE clock via the HAM activity monitor).
  - an all-ones 65th column in the value tiles makes the PV matmul emit the
    softmax denominator as PSUM row 64 for free; the epilogue uses the fast
    approximate reciprocal (~18 bits, plenty vs the 2e-2 gate).
  - everything on the PE is bf16; PSUM stays fp32.
"""

import numpy as np

C = 64
CR = 32
HW = 9216
QCH = 2304  # query rows per core
NKT = HW // 128  # 72 key tiles
R = 3  # row-tiled QK tiles per group
NG = NKT // R  # 24 key groups
NCORES = 8
QBLOCKS = [(0, 512), (512, 512), (1024, 512), (1536, 512), (2048, 256)]
PIPELINE = True  # emit QK(g+1) before PV(g)


def _ensure_ntff_hook():
    """Best-effort registration of the axon NTFF profile hook; the agent
    image's antenv package lacks axon_hooks, which would make any traced
    run crash on import instead of degrading."""
    import sys
    import types

    try:
        import antenv.axon_hooks  # noqa: F401

        return
    except ImportError:
        pass
    try:
        import antenv
        from trn_agent_boot.trn_boot import _ntff_profile_via_ctypes

        hook = _ntff_profile_via_ctypes("/opt/axon/libaxon_pjrt.so")
        mod = types.ModuleType("antenv.axon_hooks")
        _h = [hook]
        mod.get_axon_ntff_profile_hook = lambda: _h[0]
        mod.set_axon_ntff_profile_hook = lambda h: _h.__setitem__(0, h)
        sys.modules["antenv.axon_hooks"] = mod
        antenv.axon_hooks = mod
    except Exception:
        pass


def build_program(a1: float, a2: float, aa: float):
    import concourse.bacc as bacc
    import concourse.tile as tile
    from concourse import mybir

    f32 = mybir.dt.float32
    bf16 = mybir.dt.bfloat16
    AF = mybir.ActivationFunctionType

    nc = bacc.Bacc()
    xa = nc.dram_tensor("xa", [C + 1, HW], bf16, kind="ExternalInput")
    xb = nc.dram_tensor("xb", [C + 1, HW], bf16, kind="ExternalInput")
    w1r = nc.dram_tensor("w1r", [C + 1, 128], bf16, kind="ExternalInput")
    w2a = nc.dram_tensor("w2a", [C + 1, CR], bf16, kind="ExternalInput")
    waa = nc.dram_tensor("waa", [C + 1, C], bf16, kind="ExternalInput")
    out = nc.dram_tensor("out", [C, QCH], f32, kind="ExternalOutput")

    with tile.TileContext(nc) as tc:
        with (
            tc.tile_pool(name="consts", bufs=1) as consts,
            tc.tile_pool(name="big", bufs=1) as big,
            tc.tile_pool(name="ps", bufs=2, space="PSUM") as ps,
            tc.tile_pool(name="po", bufs=2, space="PSUM") as ps_o,
            tc.tile_pool(name="pt", bufs=3) as ptile,
            tc.tile_pool(name="work", bufs=2) as work,
        ):
            # --- constants / weights -------------------------------------
            w1r_sb = consts.tile([C + 1, 128], bf16, tag="w1r")
            nc.sync.dma_start(w1r_sb[:], w1r[:])
            w2a_sb = consts.tile([C + 1, CR], bf16, tag="w2a")
            nc.sync.dma_start(w2a_sb[:], w2a[:])
            waa_sb = consts.tile([C + 1, C], bf16, tag="waa")

            # --- activations in ------------------------------------------
            # no pad rows: the conv matmuls contract K=65 (half-rate on the
            # PE but tiny work) instead of a zero-padded K=128. All triggers
            # on one queue, ordered by urgency, so DMA bandwidth serves the
            # critical path first: e1 block 0, then e2 batch 0, then the rest.
            xa_sb = big.tile([C + 1, HW], bf16, tag="xa")
            xb_sb = big.tile([C + 1, HW], bf16, tag="xb")
            nc.sync.dma_start(xa_sb[:, 0:512], xa[:, 0:512])  # e1 block 0
            nc.sync.dma_start(xb_sb[:, 0:2304], xb[:, 0:2304])  # e2 blocks 0-5
            nc.sync.dma_start(waa_sb[:], waa[:])
            nc.sync.dma_start(xa_sb[:, 512:1536], xa[:, 512:1536])  # v chunk 0
            nc.sync.dma_start(xb_sb[:, 2304:4608], xb[:, 2304:4608])  # e2 6-11
            nc.sync.dma_start(xa_sb[:, 1536:3072], xa[:, 1536:3072])  # v chunk 1
            nc.sync.dma_start(xb_sb[:, 4608:HW], xb[:, 4608:HW])  # e2 12-23
            nc.sync.dma_start(xa_sb[:, 3072:HW], xa[:, 3072:HW])  # v 2-5

            e1_sb = big.tile([128, QCH], bf16, tag="e1")
            e2_sb = big.tile([96, NG * 128], bf16, tag="e2")
            v_all = big.tile([128, NKT * 65], bf16, tag="vall")
            v3 = v_all[:].rearrange("p (t c) -> p t c", c=65)
            nc.vector.memset(v3[:, :, 64:65], 1.0)

            # --- e1 = prelu(w1 @ xq + b1), replicated in 4 row groups ----
            def emit_e1(off, nq):
                pse = ps.tile([128, 1536], f32, tag="ps")
                nc.tensor.matmul(
                    pse[:, 0:nq], w1r_sb[:], xa_sb[:, off : off + nq],
                    start=True, stop=True,
                )
                ya = work.tile([128, 512], f32, tag="ya1")
                nc.vector.tensor_scalar_mul(ya[:, 0:nq], pse[:, 0:nq], a1)
                nc.vector.tensor_max(e1_sb[:, off : off + nq], ya[:, 0:nq], pse[:, 0:nq])

            # block 0 first: it is all QK(block 0) needs, so the attention
            # loop can start while the rest of the preamble still runs
            emit_e1(*QBLOCKS[0])

            # --- e2 = prelu(w2 @ xb + b2): key tile 3g+j at partitions 32j,
            # col block g of e2_sb, via col-tiled matmuls. Batch t=0 in the
            # preamble (gates QK(0)); t=1 rides in attention block 0 (only
            # gates QK(12)).
            def emit_e2(g0, ng):
                # col blocks g0..g0+ng-1 (ng <= 12)
                pse = ps.tile([128, 1536], f32, tag="ps")
                for m in range(ng):
                    g = g0 + m
                    for j in range(R):
                        kt = R * g + j
                        nc.tensor.matmul(
                            pse[32 * j : 32 * (j + 1), m * 128 : (m + 1) * 128],
                            w2a_sb[:],
                            xb_sb[:, kt * 128 : (kt + 1) * 128],
                            start=True, stop=True,
                        )
                ya = work.tile([96, 1536], f32, tag="ya2")
                nc.vector.tensor_scalar_mul(ya[:, 0 : ng * 128], pse[0:96, 0 : ng * 128], a2)
                nc.vector.tensor_max(
                    e2_sb[:, g0 * 128 : (g0 + ng) * 128],
                    ya[:, 0 : ng * 128],
                    pse[0:96, 0 : ng * 128],
                )

            emit_e2(0, 6)

            # --- v_aug tiles: [128, 65] bf16 per key tile, col 64 = ones -
            # per key tile i, psum[128,64] = xa[:, i*128:(i+1)*128]^T @ waa.
            # Emitted in 12-tile chunks: 0-1 fill the DMA-bound preamble,
            # 2-5 ride in attention block 0's tensor slack (chunk k only
            # needs to precede PV(4k-4)).
            def emit_v(grp):
                # 12 key tiles per chunk
                psv = ps.tile([128, 1536], f32, tag="ps")
                for j in range(12):
                    i = grp * 12 + j
                    nc.tensor.matmul(
                        psv[:, j * 64 : (j + 1) * 64],
                        xa_sb[:, i * 128 : (i + 1) * 128],
                        waa_sb[:],
                        start=(j % 8 == 0), stop=(j % 8 == 7 or j == 11),
                    )
                psv3 = psv[:].rearrange("p (t c) -> p t c", c=64)
                yv = work.tile([128, 768], f32, tag="yv")
                yv3 = yv[:].rearrange("p (t c) -> p t c", c=64)
                nc.vector.tensor_scalar_mul(yv[:], psv[:, 0:768], aa)
                nc.vector.tensor_max(
                    v3[:, grp * 12 : (grp + 1) * 12, 0:64], yv3[:], psv3[:, 0:12, :]
                )

            # all-ones f32 row for the PE-side reciprocal broadcast
            ones_sb = consts.tile([1, C], f32, tag="ones")
            nc.vector.memset(ones_sb[:], 1.0)

            # --- attention: per q-block, software-pipelined over key groups
            for bi, (off, nq) in enumerate(QBLOCKS):
                tpg = R
                po = ps_o.tile([C + 1, 512], f32, tag="po")
                pt_prev = None
                g_prev = -1
                for g in range(NKT // tpg):
                    # QK: 3 concurrent row-tiled K=32 matmuls, one PSUM bank
                    # each (tile j at column j*512 even when nq=256: two
                    # same-partition matmul groups in one bank hang the HW)
                    pss = ps.tile([128, 1536], f32, tag="ps")
                    for j in range(tpg):
                        kt = tpg * g + j
                        nc.tensor.matmul(
                            pss[:, j * 512 : j * 512 + nq],
                            e2_sb[
                                32 * (kt % 3) : 32 * (kt % 3 + 1),
                                (kt // 3) * 128 : (kt // 3 + 1) * 128,
                            ],
                            e1_sb[32 * (kt % 3) : 32 * (kt % 3 + 1), off : off + nq],
                            start=True, stop=True,
                        )
                    pt = ptile.tile([128, 1536], bf16, tag="pt")
                    if nq == 512:
                        nc.scalar.activation(pt[:], pss[:], AF.Exp)
                    else:
                        pss3 = pss[:].rearrange("p (t c) -> p t c", c=512)
                        pt3 = pt[:].rearrange("p (t c) -> p t c", c=512)
                        nc.scalar.activation(
                            pt3[:, :, 0:nq], pss3[:, :, 0:nq], AF.Exp
                        )
                    if bi == 0:
                        # deferred preamble work rides in block 0's tensor
                        # slack, emitted after exp(g); each piece is placed
                        # a few groups before its first consumer
                        if g == 0:
                            emit_v(0)  # before PV(0)
                        elif g == 1:
                            emit_e2(6, 6)  # before QK(6)
                        elif g == 2:
                            emit_v(1)  # before PV(4)
                        elif g == 3:
                            emit_e2(12, 6)  # before QK(12)
                        elif g == 5:
                            emit_e2(18, 6)  # before QK(18)
                        elif g in (6, 8, 10, 12):
                            emit_v(2 + (g - 6) // 2)  # chunk k before PV(4k-4)
                    if bi < 4 and g == 20:
                        # e1 for block bi+1 just before that block starts
                        emit_e1(*QBLOCKS[bi + 1])

                    def emit_pv(ptx, gx):
                        for j in range(tpg):
                            kt = tpg * gx + j
                            nc.tensor.matmul(
                                po[:, 0:nq],
                                v_all[:, kt * 65 : (kt + 1) * 65],
                                ptx[:, j * 512 : j * 512 + nq],
                                start=(kt == 0), stop=(kt == NKT - 1),
                            )

                    if PIPELINE:
                        # PV of the PREVIOUS group — emitted after QK(g) so
                        # the tensor queue never blocks on exp(g)
                        if pt_prev is not None:
                            emit_pv(pt_prev, g_prev)
                        pt_prev = pt
                        g_prev = g
                    else:
                        emit_pv(pt, g)
                if PIPELINE:
                    emit_pv(pt_prev, g_prev)
                # epilogue: out = po[0:64] / po[64] + xq   (all fp32).
                # Blocks 0-3: broadcast the reciprocal row with a 0-stride
                # DMA whose latency hides under the next block. Last block:
                # broadcast via a K=1 fp32 matmul — the PE and the psum ring
                # are idle at the tail, while a broadcast DMA would add ~4us
                # of un-hidden queue latency.
                if bi < len(QBLOCKS) - 1:
                    rec = work.tile([1, 512], f32, tag="rec")
                    nc.vector.reciprocal(rec[0:1, 0:nq], po[C : C + 1, 0:nq])
                    rb = work.tile([C, 512], f32, tag="rb")
                    rec_rep = rec[0:1, 0:nq].rearrange("a (b c) -> a b c", b=1)
                    nc.sync.dma_start(rb[:, 0:nq], rec_rep.to_broadcast((1, C, nq)))
                    osb = work.tile([C, 512], f32, tag="osb")
                    nc.vector.tensor_mul(osb[:, 0:nq], rb[:, 0:nq], po[0:C, 0:nq])
                    nc.vector.tensor_add(
                        osb[:, 0:nq], osb[:, 0:nq], xa_sb[0:C, off : off + nq]
                    )
                    nc.sync.dma_start(out[:, off : off + nq], osb[:, 0:nq])
                else:
                    # tail epilogue, split in halves and interleaved so the
                    # DVE/PE/DMA chains pipeline instead of serializing
                    hn = nq // 2
                    recs, rbps = [], []
                    for h in range(2):
                        ho = h * hn
                        rec = work.tile([1, 512], f32, tag="rec")
                        nc.vector.reciprocal(
                            rec[0:1, 0:hn], po[C : C + 1, ho : ho + hn]
                        )
                        rbp = ps.tile([128, 1536], f32, tag="ps")
                        nc.tensor.matmul(
                            rbp[0:C, 0:hn], ones_sb[:], rec[0:1, 0:hn],
                            start=True, stop=True,
                        )
                        recs.append(rec)
                        rbps.append(rbp)
                    for h in range(2):
                        ho = h * hn
                        rb = work.tile([C, 512], f32, tag="rb")
                        nc.vector.tensor_copy(rb[:, 0:hn], rbps[h][0:C, 0:hn])
                        osb = work.tile([C, 512], f32, tag="osb")
                        nc.vector.tensor_mul(
                            osb[:, 0:hn], rb[:, 0:hn], po[0:C, ho : ho + hn]
                        )
                        nc.vector.tensor_add(
                            osb[:, 0:hn], osb[:, 0:hn],
                            xa_sb[0:C, off + ho : off + ho + hn],
                        )
                        nc.sync.dma_start(
                            out[:, off + ho : off + ho + hn], osb[:, 0:hn]
                        )
    nc.finalize()
    return nc


def run(inputs: dict, trace: bool = False, tmpdir: str | None = None):
    """Build, compile and run on 8 cores; returns (output, BassKernelResults)."""
    _ensure_ntff_hook()
    from concourse.bass_utils import run_bass_kernel_spmd

    inputa = np.asarray(inputs["inputa"], dtype=np.float32)
    inputb = np.asarray(inputs["inputb"], dtype=np.float32)
    w1 = np.asarray(inputs["w1"], dtype=np.float32)
    b1 = np.asarray(inputs["b1"], dtype=np.float32)
    w2 = np.asarray(inputs["w2"], dtype=np.float32)
    b2 = np.asarray(inputs["b2"], dtype=np.float32)
    wa = np.asarray(inputs["wa"], dtype=np.float32)
    ba = np.asarray(inputs["ba"], dtype=np.float32)
    a1 = float(np.asarray(inputs["a1"]).reshape(-1)[0])
    a2 = float(np.asarray(inputs["a2"]).reshape(-1)[0])
    aa = float(np.asarray(inputs["aa"]).reshape(-1)[0])

    N, Cc, H, W = inputa.shape
    assert (N, Cc, H * W) == (2, C, HW), inputa.shape
    chunks_per_batch = NCORES // N  # 4

    import ml_dtypes

    bf = ml_dtypes.bfloat16

    xa_n = inputa.reshape(N, C, HW)
    xb_n = inputb.reshape(N, C, HW)

    def aug65(x):
        """[64, HW] -> [65, HW] bf16 with a ones row at 64."""
        p = np.empty((C + 1, x.shape[1]), np.float32)
        p[:C] = x
        p[C] = 1.0
        return p.astype(bf)

    def wpad(wt, b, rep=1):
        """[64, M] weights^T + bias row at 64; optional column replication
        for the row-tiled QK stationary layout."""
        m = wt.shape[1]
        p = np.empty((C + 1, m * rep), np.float32)
        for r in range(rep):
            p[:C, r * m : (r + 1) * m] = wt
            p[C, r * m : (r + 1) * m] = b
        return p.astype(bf)

    w1r_aug = wpad(w1.T, b1, rep=4)  # [128, 128]
    w2a_aug = wpad(w2.T, b2)  # [128, 32]
    waa_aug = wpad(wa.T, ba)  # [128, 64]

    in_maps = []
    for core in range(NCORES):
        b, chunk = divmod(core, chunks_per_batch)
        qoff = chunk * QCH
        # rotate keys/values so this core's queries are columns 0:QCH
        # (softmax over keys is invariant to the key order)
        rot = np.concatenate([xa_n[b][:, qoff:], xa_n[b][:, :qoff]], axis=1)
        rot_b = np.concatenate([xb_n[b][:, qoff:], xb_n[b][:, :qoff]], axis=1)
        in_maps.append(
            {
                "xa": aug65(rot),
                "xb": aug65(rot_b),
                "w1r": w1r_aug,
                "w2a": w2a_aug,
                "waa": waa_aug,
            }
        )

    nc = build_program(a1, a2, aa)
    res = run_bass_kernel_spmd(
        nc, in_maps, list(range(NCORES)), trace=trace, tmpdir=tmpdir
    )

    out = np.empty((N, C, HW), np.float32)
    for core in range(NCORES):
        b, chunk = divmod(core, chunks_per_batch)
        out[b, :, chunk * QCH : (chunk + 1) * QCH] = res.results[core]["out"]
    return out.reshape(N, C, H, W), res


def kernel(**inputs) -> np.ndarray:
    out, _ = run(inputs, trace=False)
    return out


# revision 37
# speedup vs baseline: 1.0098x; 1.0098x over previous
"""NonLocalAttention Trainium2 kernel (v2 — row-tiled QK, pipelined softmax).

Reference computation (N=2, C=64, CR=32, H=W=96, HW=9216):
    e1  = PReLU(w1 @ inputa + b1)   # [N,32,HW]   (queries)
    e2  = PReLU(w2 @ inputb + b2)   # [N,32,HW]   (keys)
    asm = PReLU(wa @ inputa + ba)   # [N,64,HW]   (values)
    out = softmax(e1^T e2, axis=keys) @ asm^T + inputa

Sharding: 8 cores = 2 batches x 4 query-chunks of 2304 rows. Softmax is
key-order invariant, so the host ROTATES the key/value columns per core so
that each core's query chunk is always columns 0:2304 — one SPMD program,
no per-core offsets, and no separate xq input. No collectives.

Per-core kernel (flash-style, never materializes [HW,HW]):
  - QK is ROW-TILED: the contraction is only CR=32, so three K=32 matmuls
    run CONCURRENTLY in PE row-groups 0..2 (tile_position via base_partition),
    each producing S^T for one 128-key tile. ~2.4x faster than one K=128
    matmul and needs no zero-padding of e1/e2.
  - e1 is computed with a column-replicated stationary w1r [128,128] (4
    copies of w1^T+bias) so the PReLU output lands replicated in all four
    32-partition groups, ready to be the row-tiled QK moving operand.
  - e2 is computed with COL-TILED matmuls (stationary w2 at col-group j)
    so key tile 3g+j lands directly at partitions 32j, col block g.
  - conv biases fold into the matmuls via an augmented ones-row (row 64);
    moving operands are [65,HW] from the host plus one DVE memset of rows
    65:128 (keeps K=128 full-rate contraction for the convs).
  - attention loop is software-pipelined: QK(g+1) is emitted BEFORE PV(g)
    so the tensor engine never waits on exp(g); the PV accumulator po is
    double-buffered so the softmax-divide epilogue of block b overlaps
    block b+1 (the baseline stalled 6-9us per block here, which also
    re-throttled the PE clock via the HAM activity monitor).
  - an all-ones 65th column in the value tiles makes the PV matmul emit the
    softmax denominator as PSUM row 64 for free; the epilogue uses the fast
    approximate reciprocal (~18 bits, plenty vs the 2e-2 gate).
  - everything on the PE is bf16; PSUM stays fp32.
"""

import numpy as np

C = 64
CR = 32
HW = 9216
QCH = 2304  # query rows per core
NKT = HW // 128  # 72 key tiles
R = 3  # row-tiled QK tiles per group
NG = NKT // R  # 24 key groups
NCORES = 8
QBLOCKS = [(0, 512), (512, 512), (1024, 512), (1536, 512), (2048, 256)]
PIPELINE = True  # emit QK(g+1) before PV(g)


def _ensure_ntff_hook():
    """Best-effort registration of the axon NTFF profile hook; the agent
    image's antenv package lacks axon_hooks, which would make any traced
    run crash on import instead of degrading."""
    import sys
    import types

    try:
        import antenv.axon_hooks  # noqa: F401

        return
    except ImportError:
        pass
    try:
        import antenv
        from trn_agent_boot.trn_boot import _ntff_profile_via_ctypes

        hook = _ntff_profile_via_ctypes("/opt/axon/libaxon_pjrt.so")
        mod = types.ModuleType("antenv.axon_hooks")
        _h = [hook]
        mod.get_axon_ntff_profile_hook = lambda: _h[0]
        mod.set_axon_ntff_profile_hook = lambda h: _h.__setitem__(0, h)
        sys.modules["antenv.axon_hooks"] = mod
        antenv.axon_hooks = mod
    except Exception:
        pass


def build_program(a1: float, a2: float, aa: float):
    import concourse.bacc as bacc
    import concourse.tile as tile
    from concourse import mybir

    f32 = mybir.dt.float32
    bf16 = mybir.dt.bfloat16
    AF = mybir.ActivationFunctionType

    nc = bacc.Bacc()
    xa = nc.dram_tensor("xa", [C + 1, HW], bf16, kind="ExternalInput")
    xb = nc.dram_tensor("xb", [C + 1, HW], bf16, kind="ExternalInput")
    w1r = nc.dram_tensor("w1r", [C + 1, 128], bf16, kind="ExternalInput")
    w2a = nc.dram_tensor("w2a", [C + 1, CR], bf16, kind="ExternalInput")
    waa = nc.dram_tensor("waa", [C + 1, C], bf16, kind="ExternalInput")
    out = nc.dram_tensor("out", [C, QCH], f32, kind="ExternalOutput")

    with tile.TileContext(nc) as tc:
        with (
            tc.tile_pool(name="consts", bufs=1) as consts,
            tc.tile_pool(name="big", bufs=1) as big,
            tc.tile_pool(name="ps", bufs=2, space="PSUM") as ps,
            tc.tile_pool(name="po", bufs=2, space="PSUM") as ps_o,
            tc.tile_pool(name="pt", bufs=3) as ptile,
            tc.tile_pool(name="work", bufs=2) as work,
        ):
            # --- constants / weights -------------------------------------
            w1r_sb = consts.tile([C + 1, 128], bf16, tag="w1r")
            nc.sync.dma_start(w1r_sb[:], w1r[:])
            w2a_sb = consts.tile([C + 1, CR], bf16, tag="w2a")
            nc.sync.dma_start(w2a_sb[:], w2a[:])
            waa_sb = consts.tile([C + 1, C], bf16, tag="waa")

            # --- activations in ------------------------------------------
            # no pad rows: the conv matmuls contract K=65 (half-rate on the
            # PE but tiny work) instead of a zero-padded K=128. All triggers
            # on one queue, ordered by urgency, so DMA bandwidth serves the
            # critical path first: e1 block 0, then e2 batch 0, then the rest.
            xa_sb = big.tile([C + 1, HW], bf16, tag="xa")
            xb_sb = big.tile([C + 1, HW], bf16, tag="xb")
            nc.sync.dma_start(xa_sb[:, 0:512], xa[:, 0:512])  # e1 block 0
            nc.sync.dma_start(xb_sb[:, 0:2304], xb[:, 0:2304])  # e2 blocks 0-5
            nc.sync.dma_start(waa_sb[:], waa[:])
            nc.sync.dma_start(xa_sb[:, 512:1536], xa[:, 512:1536])  # v chunk 0
            nc.sync.dma_start(xb_sb[:, 2304:4608], xb[:, 2304:4608])  # e2 6-11
            nc.sync.dma_start(xa_sb[:, 1536:3072], xa[:, 1536:3072])  # v chunk 1
            nc.sync.dma_start(xb_sb[:, 4608:HW], xb[:, 4608:HW])  # e2 12-23
            nc.sync.dma_start(xa_sb[:, 3072:HW], xa[:, 3072:HW])  # v 2-5

            e1_sb = big.tile([128, QCH], bf16, tag="e1")
            e2_sb = big.tile([96, NG * 128], bf16, tag="e2")
            v_all = big.tile([128, NKT * 65], bf16, tag="vall")
            v3 = v_all[:].rearrange("p (t c) -> p t c", c=65)
            nc.vector.memset(v3[:, :, 64:65], 1.0)

            # --- e1 = prelu(w1 @ xq + b1), replicated in 4 row groups ----
            def emit_e1(off, nq):
                # up to 3 q-blocks of 512 per psum claim
                pse = ps.tile([128, 1536], f32, tag="ps")
                for c0 in range(0, nq, 512):
                    cn = min(512, nq - c0)
                    nc.tensor.matmul(
                        pse[:, c0 : c0 + cn],
                        w1r_sb[:],
                        xa_sb[:, off + c0 : off + c0 + cn],
                        start=True, stop=True,
                    )
                ya = work.tile([128, 1536], f32, tag="ya1")
                nc.vector.tensor_scalar_mul(ya[:, 0:nq], pse[:, 0:nq], a1)
                nc.vector.tensor_max(e1_sb[:, off : off + nq], ya[:, 0:nq], pse[:, 0:nq])

            # block 0 first: it is all QK(block 0) needs, so the attention
            # loop can start while the rest of the preamble still runs
            emit_e1(0, 512)

            # --- e2 = prelu(w2 @ xb + b2): key tile 3g+j at partitions 32j,
            # col block g of e2_sb, via col-tiled matmuls. Batch t=0 in the
            # preamble (gates QK(0)); t=1 rides in attention block 0 (only
            # gates QK(12)).
            def emit_e2(g0, ng):
                # col blocks g0..g0+ng-1 (ng <= 12)
                pse = ps.tile([128, 1536], f32, tag="ps")
                for m in range(ng):
                    g = g0 + m
                    for j in range(R):
                        kt = R * g + j
                        nc.tensor.matmul(
                            pse[32 * j : 32 * (j + 1), m * 128 : (m + 1) * 128],
                            w2a_sb[:],
                            xb_sb[:, kt * 128 : (kt + 1) * 128],
                            start=True, stop=True,
                        )
                ya = work.tile([96, 1536], f32, tag="ya2")
                nc.vector.tensor_scalar_mul(ya[:, 0 : ng * 128], pse[0:96, 0 : ng * 128], a2)
                nc.vector.tensor_max(
                    e2_sb[:, g0 * 128 : (g0 + ng) * 128],
                    ya[:, 0 : ng * 128],
                    pse[0:96, 0 : ng * 128],
                )

            emit_e2(0, 6)

            # --- v_aug tiles: [128, 65] bf16 per key tile, col 64 = ones -
            # per key tile i, psum[128,64] = xa[:, i*128:(i+1)*128]^T @ waa.
            # Emitted in 12-tile chunks: 0-1 fill the DMA-bound preamble,
            # 2-5 ride in attention block 0's tensor slack (chunk k only
            # needs to precede PV(4k-4)).
            def emit_v(grp):
                # 12 key tiles per chunk
                psv = ps.tile([128, 1536], f32, tag="ps")
                for j in range(12):
                    i = grp * 12 + j
                    nc.tensor.matmul(
                        psv[:, j * 64 : (j + 1) * 64],
                        xa_sb[:, i * 128 : (i + 1) * 128],
                        waa_sb[:],
                        start=(j % 8 == 0), stop=(j % 8 == 7 or j == 11),
                    )
                psv3 = psv[:].rearrange("p (t c) -> p t c", c=64)
                yv = work.tile([128, 768], f32, tag="yv")
                yv3 = yv[:].rearrange("p (t c) -> p t c", c=64)
                nc.vector.tensor_scalar_mul(yv[:], psv[:, 0:768], aa)
                nc.vector.tensor_max(
                    v3[:, grp * 12 : (grp + 1) * 12, 0:64], yv3[:], psv3[:, 0:12, :]
                )

            # all-ones f32 row for the PE-side reciprocal broadcast
            ones_sb = consts.tile([1, C], f32, tag="ones")
            nc.vector.memset(ones_sb[:], 1.0)

            # --- attention: per q-block, software-pipelined over key groups
            for bi, (off, nq) in enumerate(QBLOCKS):
                tpg = R
                po = ps_o.tile([C + 1, 512], f32, tag="po")
                pt_prev = None
                g_prev = -1
                for g in range(NKT // tpg):
                    # QK: 3 concurrent row-tiled K=32 matmuls, one PSUM bank
                    # each (tile j at column j*512 even when nq=256: two
                    # same-partition matmul groups in one bank hang the HW)
                    pss = ps.tile([128, 1536], f32, tag="ps")
                    for j in range(tpg):
                        kt = tpg * g + j
                        nc.tensor.matmul(
                            pss[:, j * 512 : j * 512 + nq],
                            e2_sb[
                                32 * (kt % 3) : 32 * (kt % 3 + 1),
                                (kt // 3) * 128 : (kt // 3 + 1) * 128,
                            ],
                            e1_sb[32 * (kt % 3) : 32 * (kt % 3 + 1), off : off + nq],
                            start=True, stop=True,
                        )
                    pt = ptile.tile([128, 1536], bf16, tag="pt")
                    if nq == 512:
                        nc.scalar.activation(pt[:], pss[:], AF.Exp)
                    else:
                        pss3 = pss[:].rearrange("p (t c) -> p t c", c=512)
                        pt3 = pt[:].rearrange("p (t c) -> p t c", c=512)
                        nc.scalar.activation(
                            pt3[:, :, 0:nq], pss3[:, :, 0:nq], AF.Exp
                        )
                    if bi == 0:
                        # deferred preamble work rides in block 0's tensor
                        # slack, emitted after exp(g); each piece is placed
                        # a few groups before its first consumer
                        if g == 0:
                            emit_v(0)  # before PV(0)
                        elif g == 1:
                            emit_e2(6, 6)  # before QK(6)
                        elif g == 2:
                            emit_v(1)  # before PV(4)
                        elif g == 3:
                            emit_e2(12, 6)  # before QK(12)
                        elif g == 5:
                            emit_e2(18, 6)  # before QK(18)
                        elif g in (6, 8, 10, 12):
                            emit_v(2 + (g - 6) // 2)  # chunk k before PV(4k-4)
                    if bi == 0 and g == 20:
                        emit_e1(512, 1536)  # q-blocks 1-3 in one claim
                    elif bi == 1 and g == 10:
                        emit_e1(2048, 256)  # the 256 tail block

                    def emit_pv(ptx, gx):
                        for j in range(tpg):
                            kt = tpg * gx + j
                            nc.tensor.matmul(
                                po[:, 0:nq],
                                v_all[:, kt * 65 : (kt + 1) * 65],
                                ptx[:, j * 512 : j * 512 + nq],
                                start=(kt == 0), stop=(kt == NKT - 1),
                            )

                    if PIPELINE:
                        # PV of the PREVIOUS group — emitted after QK(g) so
                        # the tensor queue never blocks on exp(g)
                        if pt_prev is not None:
                            emit_pv(pt_prev, g_prev)
                        pt_prev = pt
                        g_prev = g
                    else:
                        emit_pv(pt, g)
                if PIPELINE:
                    emit_pv(pt_prev, g_prev)
                # epilogue: out = po[0:64] / po[64] + xq   (all fp32).
                # Blocks 0-3: broadcast the reciprocal row with a 0-stride
                # DMA whose latency hides under the next block. Last block:
                # broadcast via a K=1 fp32 matmul — the PE and the psum ring
                # are idle at the tail, while a broadcast DMA would add ~4us
                # of un-hidden queue latency.
                if bi < len(QBLOCKS) - 1:
                    rec = work.tile([1, 512], f32, tag="rec")
                    nc.vector.reciprocal(rec[0:1, 0:nq], po[C : C + 1, 0:nq])
                    rb = work.tile([C, 512], f32, tag="rb")
                    rec_rep = rec[0:1, 0:nq].rearrange("a (b c) -> a b c", b=1)
                    nc.sync.dma_start(rb[:, 0:nq], rec_rep.to_broadcast((1, C, nq)))
                    osb = work.tile([C, 512], f32, tag="osb")
                    nc.vector.tensor_mul(osb[:, 0:nq], rb[:, 0:nq], po[0:C, 0:nq])
                    nc.vector.tensor_add(
                        osb[:, 0:nq], osb[:, 0:nq], xa_sb[0:C, off : off + nq]
                    )
                    nc.sync.dma_start(out[:, off : off + nq], osb[:, 0:nq])
                else:
                    # tail epilogue, split in halves and interleaved so the
                    # DVE/PE/DMA chains pipeline instead of serializing
                    hn = nq // 2
                    recs, rbps = [], []
                    for h in range(2):
                        ho = h * hn
                        rec = work.tile([1, 512], f32, tag="rec")
                        nc.vector.reciprocal(
                            rec[0:1, 0:hn], po[C : C + 1, ho : ho + hn]
                        )
                        rbp = ps.tile([128, 1536], f32, tag="ps")
                        nc.tensor.matmul(
                            rbp[0:C, 0:hn], ones_sb[:], rec[0:1, 0:hn],
                            start=True, stop=True,
                        )
                        recs.append(rec)
                        rbps.append(rbp)
                    for h in range(2):
                        ho = h * hn
                        rb = work.tile([C, 512], f32, tag="rb")
                        nc.vector.tensor_copy(rb[:, 0:hn], rbps[h][0:C, 0:hn])
                        osb = work.tile([C, 512], f32, tag="osb")
                        nc.vector.tensor_mul(
                            osb[:, 0:hn], rb[:, 0:hn], po[0:C, ho : ho + hn]
                        )
                        nc.vector.tensor_add(
                            osb[:, 0:hn], osb[:, 0:hn],
                            xa_sb[0:C, off + ho : off + ho + hn],
                        )
                        nc.sync.dma_start(
                            out[:, off + ho : off + ho + hn], osb[:, 0:hn]
                        )
    nc.finalize()
    return nc


def run(inputs: dict, trace: bool = False, tmpdir: str | None = None):
    """Build, compile and run on 8 cores; returns (output, BassKernelResults)."""
    _ensure_ntff_hook()
    from concourse.bass_utils import run_bass_kernel_spmd

    inputa = np.asarray(inputs["inputa"], dtype=np.float32)
    inputb = np.asarray(inputs["inputb"], dtype=np.float32)
    w1 = np.asarray(inputs["w1"], dtype=np.float32)
    b1 = np.asarray(inputs["b1"], dtype=np.float32)
    w2 = np.asarray(inputs["w2"], dtype=np.float32)
    b2 = np.asarray(inputs["b2"], dtype=np.float32)
    wa = np.asarray(inputs["wa"], dtype=np.float32)
    ba = np.asarray(inputs["ba"], dtype=np.float32)
    a1 = float(np.asarray(inputs["a1"]).reshape(-1)[0])
    a2 = float(np.asarray(inputs["a2"]).reshape(-1)[0])
    aa = float(np.asarray(inputs["aa"]).reshape(-1)[0])

    N, Cc, H, W = inputa.shape
    assert (N, Cc, H * W) == (2, C, HW), inputa.shape
    chunks_per_batch = NCORES // N  # 4

    import ml_dtypes

    bf = ml_dtypes.bfloat16

    xa_n = inputa.reshape(N, C, HW)
    xb_n = inputb.reshape(N, C, HW)

    def aug65(x):
        """[64, HW] -> [65, HW] bf16 with a ones row at 64."""
        p = np.empty((C + 1, x.shape[1]), np.float32)
        p[:C] = x
        p[C] = 1.0
        return p.astype(bf)

    def wpad(wt, b, rep=1):
        """[64, M] weights^T + bias row at 64; optional column replication
        for the row-tiled QK stationary layout."""
        m = wt.shape[1]
        p = np.empty((C + 1, m * rep), np.float32)
        for r in range(rep):
            p[:C, r * m : (r + 1) * m] = wt
            p[C, r * m : (r + 1) * m] = b
        return p.astype(bf)

    w1r_aug = wpad(w1.T, b1, rep=4)  # [128, 128]
    w2a_aug = wpad(w2.T, b2)  # [128, 32]
    waa_aug = wpad(wa.T, ba)  # [128, 64]

    in_maps = []
    for core in range(NCORES):
        b, chunk = divmod(core, chunks_per_batch)
        qoff = chunk * QCH
        # rotate keys/values so this core's queries are columns 0:QCH
        # (softmax over keys is invariant to the key order)
        rot = np.concatenate([xa_n[b][:, qoff:], xa_n[b][:, :qoff]], axis=1)
        rot_b = np.concatenate([xb_n[b][:, qoff:], xb_n[b][:, :qoff]], axis=1)
        in_maps.append(
            {
                "xa": aug65(rot),
                "xb": aug65(rot_b),
                "w1r": w1r_aug,
                "w2a": w2a_aug,
                "waa": waa_aug,
            }
        )

    nc = build_program(a1, a2, aa)
    res = run_bass_kernel_spmd(
        nc, in_maps, list(range(NCORES)), trace=trace, tmpdir=tmpdir
    )

    out = np.empty((N, C, HW), np.float32)
    for core in range(NCORES):
        b, chunk = divmod(core, chunks_per_batch)
        out[b, :, chunk * QCH : (chunk + 1) * QCH] = res.results[core]["out"]
    return out.reshape(N, C, H, W), res


def kernel(**inputs) -> np.ndarray:
    out, _ = run(inputs, trace=False)
    return out
